# revision 8
# baseline (speedup 1.0000x reference)
"""Trainium2 kernel for nn_CMSBlockLinear (block-sparse linear layer).

Strategy: the 50%-dense random block topology (16x16 blocks) is hostile to
the 128x128 PE array, so densify the weights host-side (pure index
plumbing, no FLOPs) and run a dense [8192,2048]x[2048,8192] matmul,
token-sharded 8 ways across NeuronCores.

v2: all matmul operands in bf16 (PE streams bf16 at the same 1 elem/cell/
cycle as fp32r, but W traffic halves from 64MB to 32MB per core -- the
fp32r version was W-DMA-paced at ~147GB/s, exactly matching MM consumption).
W DMAs fetch ko-PAIRS ([128, 2x512] bf16 = 2KB per partition line) to keep
DMA line efficiency.

Per core: out[1024 tok, 8192 feat] = xT_shard.T @ W_dense
  - xT shard [128, 16, 1024] bf16 lives in SBUF; chunks are the
    stationary matmul operand.
  - W streamed in [128, 1024] bf16 tiles (2 ko-chunks, 256 KB DMAs),
    the moving operand.
  - psum [128 tok, 512 feat] fp32 accumulates over the 16 contraction
    chunks; output is written fp32 in [tokens, features] layout directly,
    so the host just concatenates the 8 shards.
"""

import sys

sys.path.insert(0, "/opt/trn_rl_repo")

import numpy as np
import ml_dtypes

T, IN_F, OUT_F = 8192, 2048, 8192
NCORES = 8
TPC = T // NCORES  # 1024 tokens per core
KO = IN_F // 128  # 16 contraction chunks of 128
KP = KO // 2  # 8 ko-pairs per W DMA
NT = OUT_F // 512  # 16 feature tiles of 512
MT = TPC // 128  # 8 token tiles of 128

_cached_nc = None


def _build_program():
    global _cached_nc
    if _cached_nc is not None:
        return _cached_nc
    from concourse import bacc, mybir, tile

    F32, BF16 = mybir.dt.float32, mybir.dt.bfloat16

    nc = bacc.Bacc(None)
    xT = nc.declare_dram_parameter("xT", [128, KO, TPC], BF16, isOutput=False)
    W = nc.declare_dram_parameter("W", [NT, KP, 128, 1024], BF16, isOutput=False)
    out = nc.declare_dram_parameter("out", [TPC, OUT_F], BF16, isOutput=True)

    with tile.TileContext(nc) as tc:
        with tc.tile_pool(name="xt", bufs=1) as xpool, \
             tc.tile_pool(name="wt", bufs=10) as wpool, \
             tc.tile_pool(name="ot", bufs=12) as opool, \
             tc.tile_pool(name="ps", bufs=1, space="PSUM") as ps:
            # HAM pre-warm: the PE sits idle ~6us while the first DMAs
            # land, and its clock gate only reaches 2.4GHz after ~3.4us of
            # sustained activity. Dummy matmuls on a zeroed tile fill the
            # idle window so the real matmuls start at full clock. They
            # write a psum slot (tag p7) whose first real use is ~1.5us
            # after the warmup drains, and start=True on the real matmul
            # overwrites whatever the warmup left there.
            wz = xpool.tile([128, 512], F32, tag="warmf", name="warm_f32")
            nc.vector.memset(wz[:], 0.0)
            warm = xpool.tile([128, 512], BF16, tag="warmr", name="warm_r")
            nc.vector.tensor_copy(warm[:], wz[:])
            wps = ps.tile([128, 512], F32, tag=f"p{MT-1}", name="warm_ps")
            for i in range(14):
                nc.tensor.matmul(wps[:], warm[:, :128], warm[:], start=True, stop=True)

            # xk0 is split in two half-tiles so the first matmuls depend on
            # a smaller DMA (64KB instead of 256KB) and start ~0.6us sooner.
            x0a = xpool.tile([128, 512], BF16, tag="x0a", name="xk0a")
            nc.scalar.dma_start(out=x0a[:], in_=xT[:, 0, 0:512])
            x0b = xpool.tile([128, 512], BF16, tag="x0b", name="xk0b")
            nc.scalar.dma_start(out=x0b[:], in_=xT[:, 0, 512:TPC])
            xts = [None]
            for ko in range(1, KO):
                # scalar HW-DGE queue: brings up ~1us faster than the gpsimd
                # SWDGE queue and the output DMAs it also carries don't start
                # until ~40us in, so there is no contention at the head.
                xk = xpool.tile([128, TPC], BF16, tag=f"x{ko}", name=f"xk{ko}")
                nc.scalar.dma_start(out=xk[:], in_=xT[:, ko, :])
                xts.append(xk)

            def xap(ko, m):
                if ko == 0:
                    t = x0a if m < 4 else x0b
                    mm = m % 4
                    return t[:, mm * 128 : (mm + 1) * 128]
                return xts[ko][:, m * 128 : (m + 1) * 128]

            for n in range(NT):
                psums = [
                    ps.tile([128, 512], F32, tag=f"p{m}", name=f"ps{n}_{m}")
                    for m in range(MT)
                ]
                for kp in range(KP):
                    if n == 0 and kp == 0:
                        # first W fetch also split so matmuls start earlier
                        wa = wpool.tile([128, 512], BF16, tag="w", name="w00a")
                        nc.sync.dma_start(out=wa[:], in_=W[0, 0][:, 0:512])
                        wb = wpool.tile([128, 512], BF16, tag="w", name="w00b")
                        nc.sync.dma_start(out=wb[:], in_=W[0, 0][:, 512:1024])
                        halves = [wa[:], wb[:]]
                    else:
                        wt = wpool.tile(
                            [128, 1024], BF16, tag="w", name=f"w{n}_{kp}"
                        )
                        nc.sync.dma_start(out=wt[:], in_=W[n, kp])
                        halves = [wt[:, 0:512], wt[:, 512:1024]]
                    for half in range(2):
                        ko = 2 * kp + half
                        for m in range(MT):
                            nc.tensor.matmul(
                                psums[m][:],
                                xap(ko, m),
                                halves[half],
                                start=(ko == 0),
                                stop=(ko == KO - 1),
                            )
                for m in range(MT):
                    # bf16 output (host upcasts; +~2e-3 rel err, within tol).
                    # Casts alternate DVE/ACT so the last n-tile's drain chain
                    # is two parallel 4-cast chains instead of one 8-cast
                    # chain; the out dma_starts ride the sync queue, which is
                    # mostly idle, instead of serializing behind ACT casts.
                    ot = opool.tile([128, 512], BF16, tag="o", name=f"o{n}_{m}")
                    if m % 2 == 0:
                        nc.vector.tensor_copy(ot[:], psums[m][:])
                    else:
                        nc.scalar.copy(ot[:], psums[m][:])
                    nc.sync.dma_start(
                        out=out[m * 128 : (m + 1) * 128, n * 512 : (n + 1) * 512],
                        in_=ot[:],
                    )
    nc.compile()
    _cached_nc = nc
    return nc


def _prep_inputs(x, values, bias, col_indices):
    x = np.ascontiguousarray(np.asarray(x), dtype=np.float32)
    values = np.ascontiguousarray(np.asarray(values), dtype=np.float32)
    bias = np.asarray(bias, dtype=np.float32)
    col_indices = np.asarray(col_indices, dtype=np.int32)

    R, K = col_indices.shape  # 512, 64
    C = IN_F // 16  # 128 column blocks

    # Scatter block values into the dense weight matrix Wd[k_in, n_out].
    Wb = np.zeros((C, R, 16, 16), np.float32)  # [c, r, i, o]
    r_idx = np.broadcast_to(np.arange(R, dtype=np.int64)[:, None], col_indices.shape)
    Wb[col_indices, r_idx] = values.transpose(0, 1, 3, 2)  # values[r,k,o,i] -> [i,o]
    Wd = Wb.transpose(0, 2, 1, 3).reshape(IN_F, OUT_F)
    Wd = Wd.astype(ml_dtypes.bfloat16)
    # [NT, KP, 128, 2, 512] -> per-partition line holds a contiguous
    # ko-pair (2KB in HBM) for DMA efficiency.
    W4 = np.ascontiguousarray(
        Wd.reshape(KP, 2, 128, NT, 512).transpose(3, 0, 2, 1, 4).reshape(NT, KP, 128, 1024)
    )

    in_maps = []
    for c in range(NCORES):
        xs = x[c * TPC : (c + 1) * TPC]  # [TPC, IN_F]
        xTc = np.ascontiguousarray(
            xs.T.reshape(KO, 128, TPC).transpose(1, 0, 2)
        ).astype(ml_dtypes.bfloat16)  # [128, KO, TPC]
        in_maps.append({"xT": xTc, "W": W4})
    return in_maps, bias


def _run(x, values, bias, col_indices, trace=False):
    from concourse.bass_utils import run_bass_kernel_spmd

    nc = _build_program()
    in_maps, bias_np = _prep_inputs(x, values, bias, col_indices)
    kwargs = {}
    if trace:
        import tempfile

        kwargs["tmpdir"] = tempfile.mkdtemp(prefix="bass_trace_")
    try:
        res = run_bass_kernel_spmd(
            nc, in_maps, list(range(NCORES)), trace=trace, **kwargs
        )
    except Exception:
        # Transient device wedges (NRT_EXEC_UNIT_UNRECOVERABLE) have been
        # observed to clear on retry.
        import time

        time.sleep(20)
        res = run_bass_kernel_spmd(
            nc, in_maps, list(range(NCORES)), trace=trace, **kwargs
        )
    out = np.concatenate(
        [res.results[c]["out"].astype(np.float32) for c in range(NCORES)], axis=0
    )
    if np.any(bias_np):
        out = out + bias_np[None, :]
    return out, res


def kernel(x, values, bias, col_indices):
    out, _ = _run(x, values, bias, col_indices)
    return out
